# revision 13
# baseline (speedup 1.0000x reference)
"""Trainium2 Bass kernel for a Tacotron-style encoder:
   embedding -> 3x (conv1d k=5 SAME + BN + ReLU) -> bidirectional LSTM (zoneout, eval).

Contract: kernel(**inputs) takes FULL unsharded inputs (as numpy arrays) and
returns the FULL [B, T, 2H] float32 output. Internally shards batch across 8
NeuronCores (data-parallel), runs a Bass/Tile kernel per core, and gathers.

Self-contained: hardcodes all shapes; does not read sibling files.

v3: fp16 front-end in (time, batch)-blocked layout (contiguous evictions),
single per-layer conv weight DMAs, recurrence with 32 segments/direction,
warm=16, software-pipelined emission across the two direction groups.
"""

import numpy as np

import concourse.bacc as bacc
import concourse.bass as bass
import concourse.tile as tile
from concourse import mybir
from concourse.bass_utils import run_bass_kernel_spmd

# Model dims (hardcoded from the problem spec)
B, T, V, E, H, F, K = 32, 512, 256, 512, 256, 512, 5
ZONEOUT = 0.1
BN_EPS = 1e-3
N_CORES = 8
B_CORE = B // N_CORES  # 4

F32 = mybir.dt.float32
F16 = mybir.dt.float16

# Gate chunk permutation: Keras order (i, f, g, o) -> device order (i, f, o, g)
_GATE_PERM = np.r_[0:2 * H, 3 * H:4 * H, 2 * H:3 * H]

# Recurrence config
SEG = 32       # segments per direction
WARM = 20      # warmup steps per segment


def build_program(Tn=T, b=B_CORE, seg=SEG, warm=WARM):
    """Build the per-core Bass program. Returns the Bacc object."""
    nc = bacc.Bacc(trn_type="TRN2", debug=False, num_devices=N_CORES)

    n_core = b * Tn
    EC = E // 128   # 4 embedding-dim chunks
    FC = F // 128   # 4 feature chunks
    VC = V // 128   # 2 vocab chunks
    GC = 4 * H // 128  # 8 gate chunks
    HC = H // 128   # 2 hidden chunks
    sl = Tn // seg        # segment length
    nslot = sl + warm     # recurrence slots per direction-group
    TP = Tn + warm        # padded xw time axis
    TB = Tn // 128        # 128-step time blocks for the front-end

    # ---- DRAM I/O (per core) ----
    tok_d = nc.dram_tensor("tokens", [n_core], F32, kind="ExternalInput")
    viota_d = nc.dram_tensor("viota", [128, VC], F32, kind="ExternalInput")
    ident_d = nc.dram_tensor("ident", [128, 128], F16, kind="ExternalInput")
    embw_d = nc.dram_tensor("embw", [128, VC, EC, 128], F16, kind="ExternalInput")
    convw_d = nc.dram_tensor("convw", [3, 128, FC, FC, K, 128], F16, kind="ExternalInput")
    cbias_d = nc.dram_tensor("cbias", [128, 3 * FC], F32, kind="ExternalInput")
    wx_d = nc.dram_tensor("wx", [128, 2, FC, GC, 128], F16, kind="ExternalInput")
    wh_d = nc.dram_tensor("wh", [128, 2, HC, GC, 128], F16, kind="ExternalInput")
    lbias_d = nc.dram_tensor("lbias", [128, 2 * GC], F32, kind="ExternalInput")
    hout_d = nc.dram_tensor("hout", [2, 128, HC, Tn, b], F16, kind="ExternalOutput")

    sig = mybir.ActivationFunctionType.Sigmoid
    tanh = mybir.ActivationFunctionType.Tanh
    mult = mybir.AluOpType.mult
    add = mybir.AluOpType.add

    with tile.TileContext(nc) as tc:
        with tc.tile_pool(name="const", bufs=1) as const, \
             tc.tile_pool(name="lstmw", bufs=1) as lstmw, \
             tc.tile_pool(name="xwp", bufs=1) as xwp, \
             tc.tile_pool(name="hbuf", bufs=1) as hbuf:

            cb = const.tile([128, 3 * FC], F32)
            nc.sync.dma_start(out=cb[:], in_=cbias_d.ap())
            lb = const.tile([128, 2 * GC], F32)
            nc.sync.dma_start(out=lb[:], in_=lbias_d.ap())
            ident = const.tile([128, 128], F16)
            nc.sync.dma_start(out=ident[:], in_=ident_d.ap())
            wh_sb = lstmw.tile([128, 2, HC, GC, 128], F16)
            nc.sync.dma_start(out=wh_sb[:], in_=wh_d.ap())
            viota = const.tile([128, VC], F32)
            nc.sync.dma_start(out=viota[:], in_=viota_d.ap())

            psb_cm = tc.tile_pool(name="psb", bufs=8, space="PSUM")
            psb = psb_cm.__enter__()

            # x layout: [128, FC, Tn+4 (time, SAME pad 2+2), b]
            with tc.tile_pool(name="xp", bufs=2) as xp:
                def fresh_x():
                    xt = xp.tile([128, FC, Tn + 4, b], F16, tag="x")
                    nc.vector.memset(xt[:, :, 0:2, :], 0.0)
                    nc.vector.memset(xt[:, :, Tn + 2:Tn + 4, :], 0.0)
                    return xt

                # ---- embedding via one-hot matmul (tokens fed t-major) ----
                with tc.tile_pool(name="embp", bufs=1) as embp:
                    embw = embp.tile([128, VC, EC, 128], F16)
                    nc.sync.dma_start(out=embw[:], in_=embw_d.ap())
                    tokb = embp.tile([128, n_core], F32)
                    tok_ap = tok_d.ap()
                    nc.sync.dma_start(
                        out=tokb[:],
                        in_=bass.AP(tensor=tok_ap.tensor, offset=0,
                                    ap=[[0, 128]] + list(tok_ap.ap)),
                    )
                    oh = embp.tile([128, VC, n_core], F16)
                    for vc in range(VC):
                        nc.vector.tensor_scalar(
                            out=oh[:, vc, :], in0=tokb[:], scalar1=viota[:, vc:vc + 1],
                            scalar2=None, op0=mybir.AluOpType.is_equal,
                        )
                    x0 = fresh_x()
                    for mc in range(EC):
                        ps4 = [psb.tile([128, 128, b], F32, tag="ps",
                                        name=f"pse{tb}") for tb in range(TB)]
                        for vc in range(VC):
                            for tb in range(TB):
                                nc.tensor.matmul(
                                    out=ps4[tb][:],
                                    lhsT=embw[:, vc, mc, :],
                                    rhs=oh[:, vc, tb * 128 * b:(tb + 1) * 128 * b],
                                    start=(vc == 0), stop=(vc == VC - 1),
                                    skip_group_check=True,
                                )
                        for tb in range(TB):
                            nc.scalar.activation(
                                out=x0[:, mc, 2 + tb * 128:2 + (tb + 1) * 128, :],
                                in_=ps4[tb][:], func=mybir.ActivationFunctionType.Copy,
                            )

                # ---- 3 conv layers (BN folded; ReLU+bias fused on eviction) ----
                xcur = x0
                with tc.tile_pool(name="cwp", bufs=2) as cwp:
                    for l in range(3):
                        wl = cwp.tile([128, FC, FC, K, 128], F16, tag="wl")
                        nc.sync.dma_start(out=wl[:], in_=convw_d.ap()[l])
                        xn = fresh_x()
                        for mc in range(FC):
                            ps4 = [psb.tile([128, 128, b], F32, tag="ps",
                                            name=f"psc{tb}") for tb in range(TB)]
                            nmm = FC * K
                            i = 0
                            for kc in range(FC):
                                for k in range(K):
                                    for tb in range(TB):
                                        nc.tensor.matmul(
                                            out=ps4[tb][:],
                                            lhsT=wl[:, mc, kc, k, :],
                                            rhs=xcur[:, kc, tb * 128 + k:tb * 128 + k + 128, :],
                                            start=(i == 0), stop=(i == nmm - 1),
                                            skip_group_check=True,
                                        )
                                    i += 1
                            for tb in range(TB):
                                nc.scalar.activation(
                                    out=xn[:, mc, 2 + tb * 128:2 + (tb + 1) * 128, :],
                                    in_=ps4[tb][:],
                                    func=mybir.ActivationFunctionType.Relu,
                                    bias=cb[:, l * FC + mc:l * FC + mc + 1],
                                )
                        xcur = xn

                # ---- LSTM input projections xw = x @ Wx + b, padded time ----
                # d=0 (fwd): time t at index warm + t; pad [0:warm) = 0
                # d=1 (bwd): time t at index t; pad [Tn:TP) = 0
                with tc.tile_pool(name="wxp", bufs=1) as wxp:
                    wx_sb = wxp.tile([128, 2, FC, GC, 128], F16)
                    nc.sync.dma_start(out=wx_sb[:], in_=wx_d.ap())
                    xw = []
                    for d in range(2):
                        xwd = xwp.tile([128, GC, TP, b], F16, tag=f"xw{d}",
                                       name=f"xw{d}")
                        if d == 0:
                            nc.vector.memset(xwd[:, :, 0:warm, :], 0.0)
                        else:
                            nc.vector.memset(xwd[:, :, Tn:TP, :], 0.0)
                        base = warm if d == 0 else 0
                        for mc in range(GC):
                            ps4 = [psb.tile([128, 128, b], F32, tag="ps",
                                            name=f"psw{tb}") for tb in range(TB)]
                            for kc in range(FC):
                                for tb in range(TB):
                                    nc.tensor.matmul(
                                        out=ps4[tb][:],
                                        lhsT=wx_sb[:, d, kc, mc, :],
                                        rhs=xcur[:, kc, 2 + tb * 128:2 + (tb + 1) * 128, :],
                                        start=(kc == 0), stop=(kc == FC - 1),
                                        skip_group_check=True,
                                    )
                            for tb in range(TB):
                                nc.scalar.activation(
                                    out=xwd[:, mc, base + tb * 128:base + (tb + 1) * 128, :],
                                    in_=ps4[tb][:],
                                    func=mybir.ActivationFunctionType.Identity,
                                    bias=lb[:, d * GC + mc:d * GC + mc + 1],
                                )
                        xw.append(xwd)

            psb_cm.__exit__(None, None, None)

            # ---- recurrence: 2 direction-groups x `seg` chains ----
            # group d, col-slice s, lane bb at slot j:
            #   d=0 processes t = sl*s + (j - warm); d=1: t = (sl-1) + warm - j + sl*s
            # Chains reading the zero pad keep exactly-zero state, so the
            # first in-range step starts from the true initial condition.
            h_sb = hbuf.tile([128, 2, HC, Tn, b], F16, name="h_sb")

            with tc.tile_pool(name="state", bufs=3) as stp, \
                 tc.tile_pool(name="ew", bufs=2) as ew, \
                 tc.tile_pool(name="psg", bufs=2, space="PSUM") as psg:

                st = []
                for d in range(2):
                    c0 = stp.tile([128, HC, seg, b], F32, tag=f"C{d}", name=f"C0_{d}")
                    nc.vector.memset(c0[:], 0.0)
                    h0 = stp.tile([128, HC, seg, b], F16, tag=f"H{d}", name=f"H0_{d}")
                    nc.vector.memset(h0[:], 0.0)
                    st.append({"C": c0, "H": h0})

                def mm_block(d, j):
                    s = st[d]
                    ps = psg.tile([128, GC, seg, b], F32, tag=f"ps{d}")
                    off = j if d == 0 else (sl - 1) + warm - j
                    xv = xw[d][:, :, off:off + (seg - 1) * sl + 1:sl, :]
                    for mc in range(GC):
                        nc.tensor.matmul(
                            out=ps[:, mc, :, :], lhsT=ident[:], rhs=xv[:, mc, :, :],
                            start=True, stop=False, skip_group_check=True)
                        for kc in range(HC):
                            nc.tensor.matmul(
                                out=ps[:, mc, :, :], lhsT=wh_sb[:, d, kc, mc, :],
                                rhs=s["H"][:, kc, :, :],
                                start=False, stop=(kc == HC - 1),
                                skip_group_check=True)
                    s["ps"] = ps

                def acts(d):
                    s = st[d]
                    S = ew.tile([128, 6, seg, b], F32, tag=f"S{d}")
                    nc.scalar.activation(out=S[:], in_=s["ps"][:, 0:6, :, :], func=sig)
                    Tg = ew.tile([128, HC, seg, b], F32, tag=f"Tg{d}")
                    nc.scalar.activation(out=Tg[:], in_=s["ps"][:, 6:8, :, :], func=tanh)
                    s["S"], s["Tg"] = S, Tg

                def vchain(d):
                    s = st[d]
                    m2 = ew.tile([128, HC, seg, b], F32, tag=f"m2{d}")
                    nc.vector.tensor_tensor(out=m2[:], in0=s["S"][:, 2:4, :, :],
                                            in1=s["C"][:], op=mult)
                    m1 = ew.tile([128, HC, seg, b], F32, tag=f"m1{d}")
                    nc.vector.tensor_tensor(out=m1[:], in0=s["S"][:, 0:2, :, :],
                                            in1=s["Tg"][:], op=mult)
                    cn = ew.tile([128, HC, seg, b], F32, tag=f"cn{d}")
                    nc.vector.scalar_tensor_tensor(
                        out=cn[:], in0=m2[:], scalar=1.0 - ZONEOUT, in1=m1[:],
                        op0=mult, op1=add)
                    s["m2"], s["cn"] = m2, cn

                def tc_act(d):
                    s = st[d]
                    TC = ew.tile([128, HC, seg, b], F32, tag=f"TC{d}")
                    nc.scalar.activation(out=TC[:], in_=s["cn"][:], func=tanh)
                    s["TC"] = TC

                def cn_update(d):
                    s = st[d]
                    Cn = stp.tile([128, HC, seg, b], F32, tag=f"C{d}", name=f"Cn{d}")
                    nc.vector.scalar_tensor_tensor(
                        out=Cn[:], in0=s["C"][:], scalar=ZONEOUT, in1=s["cn"][:],
                        op0=mult, op1=add)
                    s["C"] = Cn

                def h_update(d, j):
                    s = st[d]
                    if j >= warm:
                        po = (j - warm) if d == 0 else (sl - 1) - (j - warm)
                        hview = h_sb[:, d, :, po:po + (seg - 1) * sl + 1:sl, :]
                    else:
                        hw = ew.tile([128, HC, seg, b], F16, tag=f"hw{d}")
                        hview = hw[:]
                    nc.vector.tensor_tensor(out=hview, in0=s["S"][:, 4:6, :, :],
                                            in1=s["TC"][:], op=mult)
                    Hn = stp.tile([128, HC, seg, b], F16, tag=f"H{d}", name=f"Hn{d}")
                    nc.vector.scalar_tensor_tensor(
                        out=Hn[:], in0=s["H"][:], scalar=ZONEOUT, in1=hview,
                        op0=mult, op1=add)
                    s["H"] = Hn

                # software-pipelined emission across the two groups
                for j in range(nslot):
                    mm_block(0, j)
                    acts(0)
                    mm_block(1, j)
                    vchain(0)
                    acts(1)
                    tc_act(0)
                    vchain(1)
                    h_update(0, j)
                    tc_act(1)
                    cn_update(0)
                    h_update(1, j)
                    cn_update(1)

            for d in range(2):
                nc.sync.dma_start(out=hout_d.ap()[d], in_=h_sb[:, d, :, :, :])

    nc.compile()
    return nc


def prep_weights(emb, conv_w, conv_b, bn_gamma, bn_beta, bn_mean, bn_var,
                 lstm_wx, lstm_wh, lstm_b):
    """Host-side weight folding + layout. Returns dict of device arrays."""
    EC, FC, VC = E // 128, F // 128, V // 128
    GC, HC = 4 * H // 128, H // 128

    inv = bn_gamma / np.sqrt(bn_var + BN_EPS)              # [3, F]
    dev = {}
    dev["embw"] = np.ascontiguousarray(
        emb.reshape(VC, 128, EC, 128).transpose(1, 0, 2, 3)).astype(np.float16)

    cw = np.empty((3, 128, FC, FC, K, 128), np.float16)
    cbias = np.empty((128, 3 * FC), np.float32)
    for l in range(3):
        wf = conv_w[l] * inv[l][None, None, :]             # [K, F, F]
        # [K, FC_in, 128_in, FC_out, 128_out] -> [128_in, FC_out, FC_in, K, 128_out]
        cw[l] = wf.reshape(K, FC, 128, FC, 128).transpose(2, 3, 1, 0, 4)
        bf = (conv_b[l] - bn_mean[l]) * inv[l] + bn_beta[l]  # [F]
        cbias[:, l * FC:(l + 1) * FC] = bf.reshape(FC, 128).T
    dev["convw"] = cw
    dev["cbias"] = cbias

    wx = np.empty((128, 2, FC, GC, 128), np.float16)
    wh = np.empty((128, 2, HC, GC, 128), np.float16)
    lbias = np.empty((128, 2 * GC), np.float32)
    for d in range(2):
        wxp = lstm_wx[d][:, _GATE_PERM]                    # [F, 4H]
        wx[:, d] = wxp.reshape(FC, 128, GC, 128).transpose(1, 0, 2, 3)
        whp = (1.0 - ZONEOUT) * lstm_wh[d][:, _GATE_PERM]  # [H, 4H]
        wh[:, d] = whp.reshape(HC, 128, GC, 128).transpose(1, 0, 2, 3).astype(np.float16)
        lbias[:, d * GC:(d + 1) * GC] = lstm_b[d][_GATE_PERM].reshape(GC, 128).T
    dev["wx"] = wx
    dev["wh"] = wh
    dev["lbias"] = lbias
    dev["viota"] = np.arange(V, dtype=np.float32).reshape(VC, 128).T.copy()
    dev["ident"] = np.eye(128, dtype=np.float16)
    return dev


_CACHED_NC = None


def _get_nc():
    global _CACHED_NC
    if _CACHED_NC is None:
        _CACHED_NC = build_program()
    return _CACHED_NC


def run(inputs, trace=False, **spmd_kwargs):
    """Run on 8 cores. Returns (output [B, T, 2H] f32, BassKernelResults)."""
    nc = _get_nc()
    dev = prep_weights(
        inputs["emb"], inputs["conv_w"], inputs["conv_b"], inputs["bn_gamma"],
        inputs["bn_beta"], inputs["bn_mean"], inputs["bn_var"],
        inputs["lstm_wx"], inputs["lstm_wh"], inputs["lstm_b"])
    tokens = np.asarray(inputs["tokens"], np.int32)

    in_maps = []
    for i in range(N_CORES):
        m = dict(dev)
        # t-major per core: col index = t * b + lane
        m["tokens"] = np.ascontiguousarray(
            tokens[i * B_CORE:(i + 1) * B_CORE].T.reshape(-1).astype(np.float32))
        in_maps.append(m)

    res = run_bass_kernel_spmd(nc, in_maps, core_ids=list(range(N_CORES)),
                               trace=trace, **spmd_kwargs)

    out = np.empty((B, T, 2 * H), np.float32)
    for i in range(N_CORES):
        r = res.results[i]["hout"]            # [2, 128, HC, T, b_core] fp16
        # h[d, t, b, hc*128 + p] = r[d, p, hc, t, b]; bwd already in original time
        h = r.astype(np.float32).transpose(0, 3, 4, 2, 1).reshape(2, T, B_CORE, H)
        out[i * B_CORE:(i + 1) * B_CORE, :, 0:H] = h[0].transpose(1, 0, 2)
        out[i * B_CORE:(i + 1) * B_CORE, :, H:2 * H] = h[1].transpose(1, 0, 2)
    return out, res


def kernel(**inputs):
    return run(inputs, trace=False)[0]


# revision 14
# speedup vs baseline: 1.0456x; 1.0456x over previous
"""Trainium2 Bass kernel for a Tacotron-style encoder:
   embedding -> 3x (conv1d k=5 SAME + BN + ReLU) -> bidirectional LSTM (zoneout, eval).

Contract: kernel(**inputs) takes FULL unsharded inputs (as numpy arrays) and
returns the FULL [B, T, 2H] float32 output. Internally shards batch across 8
NeuronCores (data-parallel), runs a Bass/Tile kernel per core, and gathers.

Self-contained: hardcodes all shapes; does not read sibling files.

v3: fp16 front-end in (time, batch)-blocked layout (contiguous evictions),
single per-layer conv weight DMAs, recurrence with 32 segments/direction,
warm=16, software-pipelined emission across the two direction groups.
"""

import numpy as np

import concourse.bacc as bacc
import concourse.bass as bass
import concourse.tile as tile
from concourse import mybir
from concourse.bass_utils import run_bass_kernel_spmd

# Model dims (hardcoded from the problem spec)
B, T, V, E, H, F, K = 32, 512, 256, 512, 256, 512, 5
ZONEOUT = 0.1
BN_EPS = 1e-3
N_CORES = 8
B_CORE = B // N_CORES  # 4

F32 = mybir.dt.float32
F16 = mybir.dt.float16

# Gate chunk permutation: Keras order (i, f, g, o) -> device order (i, f, o, g)
_GATE_PERM = np.r_[0:2 * H, 3 * H:4 * H, 2 * H:3 * H]

# Recurrence config
SEG = 32       # segments per direction
WARM = 20      # warmup steps per segment


def build_program(Tn=T, b=B_CORE, seg=SEG, warm=WARM):
    """Build the per-core Bass program. Returns the Bacc object."""
    nc = bacc.Bacc(trn_type="TRN2", debug=False, num_devices=N_CORES)

    n_core = b * Tn
    EC = E // 128   # 4 embedding-dim chunks
    FC = F // 128   # 4 feature chunks
    VC = V // 128   # 2 vocab chunks
    GC = 4 * H // 128  # 8 gate chunks
    HC = H // 128   # 2 hidden chunks
    sl = Tn // seg        # segment length
    nslot = sl + warm     # recurrence slots per direction-group
    TP = Tn + warm        # padded xw time axis
    TB = Tn // 128        # 128-step time blocks for the front-end

    # ---- DRAM I/O (per core) ----
    tok_d = nc.dram_tensor("tokens", [n_core], F32, kind="ExternalInput")
    viota_d = nc.dram_tensor("viota", [128, VC], F32, kind="ExternalInput")
    ident_d = nc.dram_tensor("ident", [128, 128], F16, kind="ExternalInput")
    embw_d = nc.dram_tensor("embw", [128, VC, EC, 128], F16, kind="ExternalInput")
    convw_d = nc.dram_tensor("convw", [3, 128, FC, FC, K, 128], F16, kind="ExternalInput")
    cbias_d = nc.dram_tensor("cbias", [128, 3 * FC], F32, kind="ExternalInput")
    wx_d = nc.dram_tensor("wx", [128, 2, FC, GC, 128], F16, kind="ExternalInput")
    wh_d = nc.dram_tensor("wh", [128, 2, HC, GC, 128], F16, kind="ExternalInput")
    lbias_d = nc.dram_tensor("lbias", [128, 2 * GC], F32, kind="ExternalInput")
    hout_d = nc.dram_tensor("hout", [2, 128, HC, Tn, b], F16, kind="ExternalOutput")

    sig = mybir.ActivationFunctionType.Sigmoid
    tanh = mybir.ActivationFunctionType.Tanh
    mult = mybir.AluOpType.mult
    add = mybir.AluOpType.add

    with tile.TileContext(nc) as tc:
        with tc.tile_pool(name="const", bufs=1) as const, \
             tc.tile_pool(name="lstmw", bufs=1) as lstmw, \
             tc.tile_pool(name="xwp", bufs=1) as xwp, \
             tc.tile_pool(name="hbuf", bufs=1) as hbuf:

            cb = const.tile([128, 3 * FC], F32)
            nc.sync.dma_start(out=cb[:], in_=cbias_d.ap())
            lb = const.tile([128, 2 * GC], F32)
            nc.sync.dma_start(out=lb[:], in_=lbias_d.ap())
            ident = const.tile([128, 128], F16)
            nc.sync.dma_start(out=ident[:], in_=ident_d.ap())
            wh_sb = lstmw.tile([128, 2, HC, GC, 128], F16)
            nc.sync.dma_start(out=wh_sb[:], in_=wh_d.ap())
            viota = const.tile([128, VC], F32)
            nc.sync.dma_start(out=viota[:], in_=viota_d.ap())

            psb_cm = tc.tile_pool(name="psb", bufs=8, space="PSUM")
            psb = psb_cm.__enter__()

            # x layout: [128, FC, Tn+4 (time, SAME pad 2+2), b]
            with tc.tile_pool(name="xp", bufs=2) as xp:
                def fresh_x():
                    xt = xp.tile([128, FC, Tn + 4, b], F16, tag="x")
                    nc.vector.memset(xt[:, :, 0:2, :], 0.0)
                    nc.vector.memset(xt[:, :, Tn + 2:Tn + 4, :], 0.0)
                    return xt

                # ---- embedding via one-hot matmul (tokens fed t-major) ----
                with tc.tile_pool(name="embp", bufs=1) as embp:
                    embw = embp.tile([128, VC, EC, 128], F16)
                    nc.sync.dma_start(out=embw[:], in_=embw_d.ap())
                    tokb = embp.tile([128, n_core], F32)
                    tok_ap = tok_d.ap()
                    nc.sync.dma_start(
                        out=tokb[:],
                        in_=bass.AP(tensor=tok_ap.tensor, offset=0,
                                    ap=[[0, 128]] + list(tok_ap.ap)),
                    )
                    oh = embp.tile([128, VC, n_core], F16)
                    for vc in range(VC):
                        nc.vector.tensor_scalar(
                            out=oh[:, vc, :], in0=tokb[:], scalar1=viota[:, vc:vc + 1],
                            scalar2=None, op0=mybir.AluOpType.is_equal,
                        )
                    x0 = fresh_x()
                    for mc in range(EC):
                        ps4 = [psb.tile([128, 128, b], F32, tag="ps",
                                        name=f"pse{tb}") for tb in range(TB)]
                        for vc in range(VC):
                            for tb in range(TB):
                                nc.tensor.matmul(
                                    out=ps4[tb][:],
                                    lhsT=embw[:, vc, mc, :],
                                    rhs=oh[:, vc, tb * 128 * b:(tb + 1) * 128 * b],
                                    start=(vc == 0), stop=(vc == VC - 1),
                                    skip_group_check=True,
                                )
                        for tb in range(TB):
                            nc.scalar.activation(
                                out=x0[:, mc, 2 + tb * 128:2 + (tb + 1) * 128, :],
                                in_=ps4[tb][:], func=mybir.ActivationFunctionType.Copy,
                            )

                # ---- 3 conv layers (BN folded; ReLU+bias fused on eviction) ----
                xcur = x0
                with tc.tile_pool(name="cwp", bufs=2) as cwp:
                    for l in range(3):
                        wl = cwp.tile([128, FC, FC, K, 128], F16, tag="wl")
                        nc.sync.dma_start(out=wl[:], in_=convw_d.ap()[l])
                        xn = fresh_x()
                        for mc in range(FC):
                            ps4 = [psb.tile([128, 128, b], F32, tag="ps",
                                            name=f"psc{tb}") for tb in range(TB)]
                            nmm = FC * K
                            i = 0
                            for kc in range(FC):
                                for k in range(K):
                                    for tb in range(TB):
                                        nc.tensor.matmul(
                                            out=ps4[tb][:],
                                            lhsT=wl[:, mc, kc, k, :],
                                            rhs=xcur[:, kc, tb * 128 + k:tb * 128 + k + 128, :],
                                            start=(i == 0), stop=(i == nmm - 1),
                                            skip_group_check=True,
                                        )
                                    i += 1
                            for tb in range(TB):
                                nc.scalar.activation(
                                    out=xn[:, mc, 2 + tb * 128:2 + (tb + 1) * 128, :],
                                    in_=ps4[tb][:],
                                    func=mybir.ActivationFunctionType.Relu,
                                    bias=cb[:, l * FC + mc:l * FC + mc + 1],
                                )
                        xcur = xn

                # ---- LSTM input projections xw = x @ Wx + b, padded time ----
                # d=0 (fwd): time t at index warm + t; pad [0:warm) = 0
                # d=1 (bwd): time t at index t; pad [Tn:TP) = 0
                with tc.tile_pool(name="wxp", bufs=1) as wxp:
                    wx_sb = wxp.tile([128, 2, FC, GC, 128], F16)
                    nc.sync.dma_start(out=wx_sb[:], in_=wx_d.ap())
                    xw = []
                    for d in range(2):
                        xwd = xwp.tile([128, GC, TP, b], F16, tag=f"xw{d}",
                                       name=f"xw{d}")
                        if d == 0:
                            nc.vector.memset(xwd[:, :, 0:warm, :], 0.0)
                        else:
                            nc.vector.memset(xwd[:, :, Tn:TP, :], 0.0)
                        base = warm if d == 0 else 0
                        for mc in range(GC):
                            ps4 = [psb.tile([128, 128, b], F32, tag="ps",
                                            name=f"psw{tb}") for tb in range(TB)]
                            for kc in range(FC):
                                for tb in range(TB):
                                    nc.tensor.matmul(
                                        out=ps4[tb][:],
                                        lhsT=wx_sb[:, d, kc, mc, :],
                                        rhs=xcur[:, kc, 2 + tb * 128:2 + (tb + 1) * 128, :],
                                        start=(kc == 0), stop=(kc == FC - 1),
                                        skip_group_check=True,
                                    )
                            for tb in range(TB):
                                nc.scalar.activation(
                                    out=xwd[:, mc, base + tb * 128:base + (tb + 1) * 128, :],
                                    in_=ps4[tb][:],
                                    func=mybir.ActivationFunctionType.Identity,
                                    bias=lb[:, d * GC + mc:d * GC + mc + 1],
                                )
                        xw.append(xwd)

            psb_cm.__exit__(None, None, None)

            # ---- recurrence: 2 direction-groups x `seg` chains ----
            # group d, col-slice s, lane bb at slot j:
            #   d=0 processes t = sl*s + (j - warm); d=1: t = (sl-1) + warm - j + sl*s
            # Chains reading the zero pad keep exactly-zero state, so the
            # first in-range step starts from the true initial condition.
            h_sb = hbuf.tile([128, 2, HC, Tn, b], F16, name="h_sb")

            with tc.tile_pool(name="state", bufs=3) as stp, \
                 tc.tile_pool(name="ew", bufs=2) as ew, \
                 tc.tile_pool(name="psg", bufs=2, space="PSUM") as psg:

                st = []
                for d in range(2):
                    c0 = stp.tile([128, HC, seg, b], F32, tag=f"C{d}", name=f"C0_{d}")
                    nc.vector.memset(c0[:], 0.0)
                    h0 = stp.tile([128, HC, seg, b], F16, tag=f"H{d}", name=f"H0_{d}")
                    nc.vector.memset(h0[:], 0.0)
                    st.append({"C": c0, "H": h0})

                def mm_block(d, j):
                    s = st[d]
                    ps = psg.tile([128, GC, seg, b], F32, tag=f"ps{d}")
                    off = j if d == 0 else (sl - 1) + warm - j
                    xv = xw[d][:, :, off:off + (seg - 1) * sl + 1:sl, :]
                    for mc in range(GC):
                        nc.tensor.matmul(
                            out=ps[:, mc, :, :], lhsT=ident[:], rhs=xv[:, mc, :, :],
                            start=True, stop=False, skip_group_check=True)
                        for kc in range(HC):
                            nc.tensor.matmul(
                                out=ps[:, mc, :, :], lhsT=wh_sb[:, d, kc, mc, :],
                                rhs=s["H"][:, kc, :, :],
                                start=False, stop=(kc == HC - 1),
                                skip_group_check=True)
                    s["ps"] = ps

                def acts(d):
                    s = st[d]
                    S = ew.tile([128, 6, seg, b], F16, tag=f"S{d}")
                    nc.scalar.activation(out=S[:], in_=s["ps"][:, 0:6, :, :], func=sig)
                    Tg = ew.tile([128, HC, seg, b], F16, tag=f"Tg{d}")
                    nc.scalar.activation(out=Tg[:], in_=s["ps"][:, 6:8, :, :], func=tanh)
                    s["S"], s["Tg"] = S, Tg

                def vchain(d):
                    s = st[d]
                    m2 = ew.tile([128, HC, seg, b], F32, tag=f"m2{d}")
                    nc.vector.tensor_tensor(out=m2[:], in0=s["S"][:, 2:4, :, :],
                                            in1=s["C"][:], op=mult)
                    m1 = ew.tile([128, HC, seg, b], F16, tag=f"m1{d}")
                    nc.vector.tensor_tensor(out=m1[:], in0=s["S"][:, 0:2, :, :],
                                            in1=s["Tg"][:], op=mult)
                    cn = ew.tile([128, HC, seg, b], F32, tag=f"cn{d}")
                    nc.vector.scalar_tensor_tensor(
                        out=cn[:], in0=m2[:], scalar=1.0 - ZONEOUT, in1=m1[:],
                        op0=mult, op1=add)
                    s["m2"], s["cn"] = m2, cn

                def tc_act(d):
                    s = st[d]
                    TC = ew.tile([128, HC, seg, b], F16, tag=f"TC{d}")
                    nc.scalar.activation(out=TC[:], in_=s["cn"][:], func=tanh)
                    s["TC"] = TC

                def cn_update(d):
                    s = st[d]
                    Cn = stp.tile([128, HC, seg, b], F32, tag=f"C{d}", name=f"Cn{d}")
                    nc.vector.scalar_tensor_tensor(
                        out=Cn[:], in0=s["C"][:], scalar=ZONEOUT, in1=s["cn"][:],
                        op0=mult, op1=add)
                    s["C"] = Cn

                def h_update(d, j):
                    s = st[d]
                    if j >= warm:
                        po = (j - warm) if d == 0 else (sl - 1) - (j - warm)
                        hview = h_sb[:, d, :, po:po + (seg - 1) * sl + 1:sl, :]
                    else:
                        hw = ew.tile([128, HC, seg, b], F16, tag=f"hw{d}")
                        hview = hw[:]
                    nc.vector.tensor_tensor(out=hview, in0=s["S"][:, 4:6, :, :],
                                            in1=s["TC"][:], op=mult)
                    Hn = stp.tile([128, HC, seg, b], F16, tag=f"H{d}", name=f"Hn{d}")
                    nc.vector.scalar_tensor_tensor(
                        out=Hn[:], in0=s["H"][:], scalar=ZONEOUT, in1=hview,
                        op0=mult, op1=add)
                    s["H"] = Hn

                # software-pipelined emission across the two groups
                for j in range(nslot):
                    mm_block(0, j)
                    acts(0)
                    mm_block(1, j)
                    vchain(0)
                    acts(1)
                    tc_act(0)
                    vchain(1)
                    h_update(0, j)
                    tc_act(1)
                    cn_update(0)
                    h_update(1, j)
                    cn_update(1)

            for d in range(2):
                nc.sync.dma_start(out=hout_d.ap()[d], in_=h_sb[:, d, :, :, :])

    nc.compile()
    return nc


def prep_weights(emb, conv_w, conv_b, bn_gamma, bn_beta, bn_mean, bn_var,
                 lstm_wx, lstm_wh, lstm_b):
    """Host-side weight folding + layout. Returns dict of device arrays."""
    EC, FC, VC = E // 128, F // 128, V // 128
    GC, HC = 4 * H // 128, H // 128

    inv = bn_gamma / np.sqrt(bn_var + BN_EPS)              # [3, F]
    dev = {}
    dev["embw"] = np.ascontiguousarray(
        emb.reshape(VC, 128, EC, 128).transpose(1, 0, 2, 3)).astype(np.float16)

    cw = np.empty((3, 128, FC, FC, K, 128), np.float16)
    cbias = np.empty((128, 3 * FC), np.float32)
    for l in range(3):
        wf = conv_w[l] * inv[l][None, None, :]             # [K, F, F]
        # [K, FC_in, 128_in, FC_out, 128_out] -> [128_in, FC_out, FC_in, K, 128_out]
        cw[l] = wf.reshape(K, FC, 128, FC, 128).transpose(2, 3, 1, 0, 4)
        bf = (conv_b[l] - bn_mean[l]) * inv[l] + bn_beta[l]  # [F]
        cbias[:, l * FC:(l + 1) * FC] = bf.reshape(FC, 128).T
    dev["convw"] = cw
    dev["cbias"] = cbias

    wx = np.empty((128, 2, FC, GC, 128), np.float16)
    wh = np.empty((128, 2, HC, GC, 128), np.float16)
    lbias = np.empty((128, 2 * GC), np.float32)
    for d in range(2):
        wxp = lstm_wx[d][:, _GATE_PERM]                    # [F, 4H]
        wx[:, d] = wxp.reshape(FC, 128, GC, 128).transpose(1, 0, 2, 3)
        whp = (1.0 - ZONEOUT) * lstm_wh[d][:, _GATE_PERM]  # [H, 4H]
        wh[:, d] = whp.reshape(HC, 128, GC, 128).transpose(1, 0, 2, 3).astype(np.float16)
        lbias[:, d * GC:(d + 1) * GC] = lstm_b[d][_GATE_PERM].reshape(GC, 128).T
    dev["wx"] = wx
    dev["wh"] = wh
    dev["lbias"] = lbias
    dev["viota"] = np.arange(V, dtype=np.float32).reshape(VC, 128).T.copy()
    dev["ident"] = np.eye(128, dtype=np.float16)
    return dev


_CACHED_NC = None


def _get_nc():
    global _CACHED_NC
    if _CACHED_NC is None:
        _CACHED_NC = build_program()
    return _CACHED_NC


def run(inputs, trace=False, **spmd_kwargs):
    """Run on 8 cores. Returns (output [B, T, 2H] f32, BassKernelResults)."""
    nc = _get_nc()
    dev = prep_weights(
        inputs["emb"], inputs["conv_w"], inputs["conv_b"], inputs["bn_gamma"],
        inputs["bn_beta"], inputs["bn_mean"], inputs["bn_var"],
        inputs["lstm_wx"], inputs["lstm_wh"], inputs["lstm_b"])
    tokens = np.asarray(inputs["tokens"], np.int32)

    in_maps = []
    for i in range(N_CORES):
        m = dict(dev)
        # t-major per core: col index = t * b + lane
        m["tokens"] = np.ascontiguousarray(
            tokens[i * B_CORE:(i + 1) * B_CORE].T.reshape(-1).astype(np.float32))
        in_maps.append(m)

    res = run_bass_kernel_spmd(nc, in_maps, core_ids=list(range(N_CORES)),
                               trace=trace, **spmd_kwargs)

    out = np.empty((B, T, 2 * H), np.float32)
    for i in range(N_CORES):
        r = res.results[i]["hout"]            # [2, 128, HC, T, b_core] fp16
        # h[d, t, b, hc*128 + p] = r[d, p, hc, t, b]; bwd already in original time
        h = r.astype(np.float32).transpose(0, 3, 4, 2, 1).reshape(2, T, B_CORE, H)
        out[i * B_CORE:(i + 1) * B_CORE, :, 0:H] = h[0].transpose(1, 0, 2)
        out[i * B_CORE:(i + 1) * B_CORE, :, H:2 * H] = h[1].transpose(1, 0, 2)
    return out, res


def kernel(**inputs):
    return run(inputs, trace=False)[0]


# revision 15
# speedup vs baseline: 1.0472x; 1.0015x over previous
"""Trainium2 Bass kernel for a Tacotron-style encoder:
   embedding -> 3x (conv1d k=5 SAME + BN + ReLU) -> bidirectional LSTM (zoneout, eval).

Contract: kernel(**inputs) takes FULL unsharded inputs (as numpy arrays) and
returns the FULL [B, T, 2H] float32 output. Internally shards batch across 8
NeuronCores (data-parallel), runs a Bass/Tile kernel per core, and gathers.

Self-contained: hardcodes all shapes; does not read sibling files.

v3: fp16 front-end in (time, batch)-blocked layout (contiguous evictions),
single per-layer conv weight DMAs, recurrence with 32 segments/direction,
warm=16, software-pipelined emission across the two direction groups.
"""

import numpy as np
import ml_dtypes
BF = ml_dtypes.bfloat16

import concourse.bacc as bacc
import concourse.bass as bass
import concourse.tile as tile
from concourse import mybir
from concourse.bass_utils import run_bass_kernel_spmd

# Model dims (hardcoded from the problem spec)
B, T, V, E, H, F, K = 32, 512, 256, 512, 256, 512, 5
ZONEOUT = 0.1
BN_EPS = 1e-3
N_CORES = 8
B_CORE = B // N_CORES  # 4

F32 = mybir.dt.float32
F16 = mybir.dt.float16
BF16 = mybir.dt.bfloat16

# Gate chunk permutation: Keras order (i, f, g, o) -> device order (i, f, o, g)
_GATE_PERM = np.r_[0:2 * H, 3 * H:4 * H, 2 * H:3 * H]

# Recurrence config
SEG = 32       # segments per direction
WARM = 20      # warmup steps per segment


def build_program(Tn=T, b=B_CORE, seg=SEG, warm=WARM):
    """Build the per-core Bass program. Returns the Bacc object."""
    nc = bacc.Bacc(trn_type="TRN2", debug=False, num_devices=N_CORES)

    n_core = b * Tn
    EC = E // 128   # 4 embedding-dim chunks
    FC = F // 128   # 4 feature chunks
    VC = V // 128   # 2 vocab chunks
    GC = 4 * H // 128  # 8 gate chunks
    HC = H // 128   # 2 hidden chunks
    sl = Tn // seg        # segment length
    nslot = sl + warm     # recurrence slots per direction-group
    TP = Tn + warm        # padded xw time axis
    TB = Tn // 128        # 128-step time blocks for the front-end

    # ---- DRAM I/O (per core) ----
    tok_d = nc.dram_tensor("tokens", [n_core], F32, kind="ExternalInput")
    viota_d = nc.dram_tensor("viota", [128, VC], F32, kind="ExternalInput")
    ident_d = nc.dram_tensor("ident", [128, 128], F16, kind="ExternalInput")
    embw_d = nc.dram_tensor("embw", [128, VC, EC, 128], BF16, kind="ExternalInput")
    convw_d = nc.dram_tensor("convw", [3, 128, FC, FC, K, 128], BF16, kind="ExternalInput")
    cbias_d = nc.dram_tensor("cbias", [128, 3 * FC], F32, kind="ExternalInput")
    wx_d = nc.dram_tensor("wx", [128, 2, FC, GC, 128], BF16, kind="ExternalInput")
    wh_d = nc.dram_tensor("wh", [128, 2, HC, GC, 128], F16, kind="ExternalInput")
    lbias_d = nc.dram_tensor("lbias", [128, 2 * GC], F32, kind="ExternalInput")
    hout_d = nc.dram_tensor("hout", [2, 128, HC, Tn, b], F16, kind="ExternalOutput")

    sig = mybir.ActivationFunctionType.Sigmoid
    tanh = mybir.ActivationFunctionType.Tanh
    mult = mybir.AluOpType.mult
    add = mybir.AluOpType.add

    with tile.TileContext(nc) as tc:
        with tc.tile_pool(name="const", bufs=1) as const, \
             tc.tile_pool(name="lstmw", bufs=1) as lstmw, \
             tc.tile_pool(name="xwp", bufs=1) as xwp, \
             tc.tile_pool(name="hbuf", bufs=1) as hbuf:

            cb = const.tile([128, 3 * FC], F32)
            nc.sync.dma_start(out=cb[:], in_=cbias_d.ap())
            lb = const.tile([128, 2 * GC], F32)
            nc.sync.dma_start(out=lb[:], in_=lbias_d.ap())
            ident = const.tile([128, 128], F16)
            nc.sync.dma_start(out=ident[:], in_=ident_d.ap())
            wh_sb = lstmw.tile([128, 2, HC, GC, 128], F16)
            nc.sync.dma_start(out=wh_sb[:], in_=wh_d.ap())
            viota = const.tile([128, VC], F32)
            nc.sync.dma_start(out=viota[:], in_=viota_d.ap())

            psb_cm = tc.tile_pool(name="psb", bufs=8, space="PSUM")
            psb = psb_cm.__enter__()

            # x layout: [128, FC, Tn+4 (time, SAME pad 2+2), b]
            with tc.tile_pool(name="xp", bufs=2) as xp:
                def fresh_x():
                    xt = xp.tile([128, FC, Tn + 4, b], BF16, tag="x")
                    nc.vector.memset(xt[:, :, 0:2, :], 0.0)
                    nc.vector.memset(xt[:, :, Tn + 2:Tn + 4, :], 0.0)
                    return xt

                # ---- embedding via one-hot matmul (tokens fed t-major) ----
                with tc.tile_pool(name="embp", bufs=1) as embp:
                    embw = embp.tile([128, VC, EC, 128], BF16)
                    nc.sync.dma_start(out=embw[:], in_=embw_d.ap())
                    tokb = embp.tile([128, n_core], F32)
                    tok_ap = tok_d.ap()
                    nc.sync.dma_start(
                        out=tokb[:],
                        in_=bass.AP(tensor=tok_ap.tensor, offset=0,
                                    ap=[[0, 128]] + list(tok_ap.ap)),
                    )
                    oh = embp.tile([128, VC, n_core], BF16)
                    for vc in range(VC):
                        nc.vector.tensor_scalar(
                            out=oh[:, vc, :], in0=tokb[:], scalar1=viota[:, vc:vc + 1],
                            scalar2=None, op0=mybir.AluOpType.is_equal,
                        )
                    x0 = fresh_x()
                    for mc in range(EC):
                        ps4 = [psb.tile([128, 128, b], F32, tag="ps",
                                        name=f"pse{tb}") for tb in range(TB)]
                        for vc in range(VC):
                            for tb in range(TB):
                                nc.tensor.matmul(
                                    out=ps4[tb][:],
                                    lhsT=embw[:, vc, mc, :],
                                    rhs=oh[:, vc, tb * 128 * b:(tb + 1) * 128 * b],
                                    start=(vc == 0), stop=(vc == VC - 1),
                                    skip_group_check=True,
                                )
                        for tb in range(TB):
                            nc.scalar.activation(
                                out=x0[:, mc, 2 + tb * 128:2 + (tb + 1) * 128, :],
                                in_=ps4[tb][:], func=mybir.ActivationFunctionType.Copy,
                            )

                # ---- 3 conv layers (BN folded; ReLU+bias fused on eviction) ----
                xcur = x0
                with tc.tile_pool(name="cwp", bufs=2) as cwp:
                    for l in range(3):
                        wl = cwp.tile([128, FC, FC, K, 128], BF16, tag="wl")
                        nc.sync.dma_start(out=wl[:], in_=convw_d.ap()[l])
                        xn = fresh_x()
                        for mc in range(FC):
                            ps4 = [psb.tile([128, 128, b], F32, tag="ps",
                                            name=f"psc{tb}") for tb in range(TB)]
                            nmm = FC * K
                            i = 0
                            for kc in range(FC):
                                for k in range(K):
                                    for tb in range(TB):
                                        nc.tensor.matmul(
                                            out=ps4[tb][:],
                                            lhsT=wl[:, mc, kc, k, :],
                                            rhs=xcur[:, kc, tb * 128 + k:tb * 128 + k + 128, :],
                                            start=(i == 0), stop=(i == nmm - 1),
                                            skip_group_check=True,
                                        )
                                    i += 1
                            for tb in range(TB):
                                nc.scalar.activation(
                                    out=xn[:, mc, 2 + tb * 128:2 + (tb + 1) * 128, :],
                                    in_=ps4[tb][:],
                                    func=mybir.ActivationFunctionType.Relu,
                                    bias=cb[:, l * FC + mc:l * FC + mc + 1],
                                )
                        xcur = xn

                # ---- LSTM input projections xw = x @ Wx + b, padded time ----
                # d=0 (fwd): time t at index warm + t; pad [0:warm) = 0
                # d=1 (bwd): time t at index t; pad [Tn:TP) = 0
                with tc.tile_pool(name="wxp", bufs=1) as wxp:
                    wx_sb = wxp.tile([128, 2, FC, GC, 128], BF16)
                    nc.sync.dma_start(out=wx_sb[:], in_=wx_d.ap())
                    xw = []
                    for d in range(2):
                        xwd = xwp.tile([128, GC, TP, b], F16, tag=f"xw{d}",
                                       name=f"xw{d}")
                        if d == 0:
                            nc.vector.memset(xwd[:, :, 0:warm, :], 0.0)
                        else:
                            nc.vector.memset(xwd[:, :, Tn:TP, :], 0.0)
                        base = warm if d == 0 else 0
                        for mc in range(GC):
                            ps4 = [psb.tile([128, 128, b], F32, tag="ps",
                                            name=f"psw{tb}") for tb in range(TB)]
                            for kc in range(FC):
                                for tb in range(TB):
                                    nc.tensor.matmul(
                                        out=ps4[tb][:],
                                        lhsT=wx_sb[:, d, kc, mc, :],
                                        rhs=xcur[:, kc, 2 + tb * 128:2 + (tb + 1) * 128, :],
                                        start=(kc == 0), stop=(kc == FC - 1),
                                        skip_group_check=True,
                                    )
                            for tb in range(TB):
                                nc.scalar.activation(
                                    out=xwd[:, mc, base + tb * 128:base + (tb + 1) * 128, :],
                                    in_=ps4[tb][:],
                                    func=mybir.ActivationFunctionType.Identity,
                                    bias=lb[:, d * GC + mc:d * GC + mc + 1],
                                )
                        xw.append(xwd)

            psb_cm.__exit__(None, None, None)

            # ---- recurrence: 2 direction-groups x `seg` chains ----
            # group d, col-slice s, lane bb at slot j:
            #   d=0 processes t = sl*s + (j - warm); d=1: t = (sl-1) + warm - j + sl*s
            # Chains reading the zero pad keep exactly-zero state, so the
            # first in-range step starts from the true initial condition.
            h_sb = hbuf.tile([128, 2, HC, Tn, b], F16, name="h_sb")

            with tc.tile_pool(name="state", bufs=3) as stp, \
                 tc.tile_pool(name="ew", bufs=2) as ew, \
                 tc.tile_pool(name="psg", bufs=2, space="PSUM") as psg:

                st = []
                for d in range(2):
                    c0 = stp.tile([128, HC, seg, b], F32, tag=f"C{d}", name=f"C0_{d}")
                    nc.vector.memset(c0[:], 0.0)
                    h0 = stp.tile([128, HC, seg, b], F16, tag=f"H{d}", name=f"H0_{d}")
                    nc.vector.memset(h0[:], 0.0)
                    st.append({"C": c0, "H": h0})

                def mm_block(d, j):
                    s = st[d]
                    ps = psg.tile([128, GC, seg, b], F32, tag=f"ps{d}")
                    off = j if d == 0 else (sl - 1) + warm - j
                    xv = xw[d][:, :, off:off + (seg - 1) * sl + 1:sl, :]
                    for mc in range(GC):
                        nc.tensor.matmul(
                            out=ps[:, mc, :, :], lhsT=ident[:], rhs=xv[:, mc, :, :],
                            start=True, stop=False, skip_group_check=True)
                        for kc in range(HC):
                            nc.tensor.matmul(
                                out=ps[:, mc, :, :], lhsT=wh_sb[:, d, kc, mc, :],
                                rhs=s["H"][:, kc, :, :],
                                start=False, stop=(kc == HC - 1),
                                skip_group_check=True)
                    s["ps"] = ps

                def acts(d):
                    s = st[d]
                    S = ew.tile([128, 6, seg, b], F16, tag=f"S{d}")
                    nc.scalar.activation(out=S[:], in_=s["ps"][:, 0:6, :, :], func=sig)
                    Tg = ew.tile([128, HC, seg, b], F16, tag=f"Tg{d}")
                    nc.scalar.activation(out=Tg[:], in_=s["ps"][:, 6:8, :, :], func=tanh)
                    s["S"], s["Tg"] = S, Tg

                def vchain(d):
                    s = st[d]
                    m2 = ew.tile([128, HC, seg, b], F32, tag=f"m2{d}")
                    nc.vector.tensor_tensor(out=m2[:], in0=s["S"][:, 2:4, :, :],
                                            in1=s["C"][:], op=mult)
                    m1 = ew.tile([128, HC, seg, b], F16, tag=f"m1{d}")
                    nc.vector.tensor_tensor(out=m1[:], in0=s["S"][:, 0:2, :, :],
                                            in1=s["Tg"][:], op=mult)
                    cn = ew.tile([128, HC, seg, b], F32, tag=f"cn{d}")
                    nc.vector.scalar_tensor_tensor(
                        out=cn[:], in0=m2[:], scalar=1.0 - ZONEOUT, in1=m1[:],
                        op0=mult, op1=add)
                    s["m2"], s["cn"] = m2, cn

                def tc_act(d):
                    s = st[d]
                    TC = ew.tile([128, HC, seg, b], F16, tag=f"TC{d}")
                    nc.scalar.activation(out=TC[:], in_=s["cn"][:], func=tanh)
                    s["TC"] = TC

                def cn_update(d):
                    s = st[d]
                    Cn = stp.tile([128, HC, seg, b], F32, tag=f"C{d}", name=f"Cn{d}")
                    nc.vector.scalar_tensor_tensor(
                        out=Cn[:], in0=s["C"][:], scalar=ZONEOUT, in1=s["cn"][:],
                        op0=mult, op1=add)
                    s["C"] = Cn

                def h_update(d, j):
                    s = st[d]
                    if j >= warm:
                        po = (j - warm) if d == 0 else (sl - 1) - (j - warm)
                        hview = h_sb[:, d, :, po:po + (seg - 1) * sl + 1:sl, :]
                    else:
                        hw = ew.tile([128, HC, seg, b], F16, tag=f"hw{d}")
                        hview = hw[:]
                    nc.vector.tensor_tensor(out=hview, in0=s["S"][:, 4:6, :, :],
                                            in1=s["TC"][:], op=mult)
                    Hn = stp.tile([128, HC, seg, b], F16, tag=f"H{d}", name=f"Hn{d}")
                    nc.vector.scalar_tensor_tensor(
                        out=Hn[:], in0=s["H"][:], scalar=ZONEOUT, in1=hview,
                        op0=mult, op1=add)
                    s["H"] = Hn

                # software-pipelined emission across the two groups
                for j in range(nslot):
                    mm_block(0, j)
                    acts(0)
                    mm_block(1, j)
                    vchain(0)
                    acts(1)
                    tc_act(0)
                    vchain(1)
                    h_update(0, j)
                    tc_act(1)
                    cn_update(0)
                    h_update(1, j)
                    cn_update(1)

            for d in range(2):
                nc.sync.dma_start(out=hout_d.ap()[d], in_=h_sb[:, d, :, :, :])

    nc.compile()
    return nc


def prep_weights(emb, conv_w, conv_b, bn_gamma, bn_beta, bn_mean, bn_var,
                 lstm_wx, lstm_wh, lstm_b):
    """Host-side weight folding + layout. Returns dict of device arrays."""
    EC, FC, VC = E // 128, F // 128, V // 128
    GC, HC = 4 * H // 128, H // 128

    inv = bn_gamma / np.sqrt(bn_var + BN_EPS)              # [3, F]
    dev = {}
    dev["embw"] = np.ascontiguousarray(
        emb.reshape(VC, 128, EC, 128).transpose(1, 0, 2, 3)).astype(BF)

    cw = np.empty((3, 128, FC, FC, K, 128), BF)
    cbias = np.empty((128, 3 * FC), np.float32)
    for l in range(3):
        wf = conv_w[l] * inv[l][None, None, :]             # [K, F, F]
        # [K, FC_in, 128_in, FC_out, 128_out] -> [128_in, FC_out, FC_in, K, 128_out]
        cw[l] = wf.reshape(K, FC, 128, FC, 128).transpose(2, 3, 1, 0, 4)
        bf = (conv_b[l] - bn_mean[l]) * inv[l] + bn_beta[l]  # [F]
        cbias[:, l * FC:(l + 1) * FC] = bf.reshape(FC, 128).T
    dev["convw"] = cw
    dev["cbias"] = cbias

    wx = np.empty((128, 2, FC, GC, 128), BF)
    wh = np.empty((128, 2, HC, GC, 128), np.float16)
    lbias = np.empty((128, 2 * GC), np.float32)
    for d in range(2):
        wxp = lstm_wx[d][:, _GATE_PERM]                    # [F, 4H]
        wx[:, d] = wxp.reshape(FC, 128, GC, 128).transpose(1, 0, 2, 3)
        whp = (1.0 - ZONEOUT) * lstm_wh[d][:, _GATE_PERM]  # [H, 4H]
        wh[:, d] = whp.reshape(HC, 128, GC, 128).transpose(1, 0, 2, 3).astype(np.float16)
        lbias[:, d * GC:(d + 1) * GC] = lstm_b[d][_GATE_PERM].reshape(GC, 128).T
    dev["wx"] = wx
    dev["wh"] = wh
    dev["lbias"] = lbias
    dev["viota"] = np.arange(V, dtype=np.float32).reshape(VC, 128).T.copy()
    dev["ident"] = np.eye(128, dtype=np.float16)
    return dev


_CACHED_NC = None


def _get_nc():
    global _CACHED_NC
    if _CACHED_NC is None:
        _CACHED_NC = build_program()
    return _CACHED_NC


def run(inputs, trace=False, **spmd_kwargs):
    """Run on 8 cores. Returns (output [B, T, 2H] f32, BassKernelResults)."""
    nc = _get_nc()
    dev = prep_weights(
        inputs["emb"], inputs["conv_w"], inputs["conv_b"], inputs["bn_gamma"],
        inputs["bn_beta"], inputs["bn_mean"], inputs["bn_var"],
        inputs["lstm_wx"], inputs["lstm_wh"], inputs["lstm_b"])
    tokens = np.asarray(inputs["tokens"], np.int32)

    in_maps = []
    for i in range(N_CORES):
        m = dict(dev)
        # t-major per core: col index = t * b + lane
        m["tokens"] = np.ascontiguousarray(
            tokens[i * B_CORE:(i + 1) * B_CORE].T.reshape(-1).astype(np.float32))
        in_maps.append(m)

    res = run_bass_kernel_spmd(nc, in_maps, core_ids=list(range(N_CORES)),
                               trace=trace, **spmd_kwargs)

    out = np.empty((B, T, 2 * H), np.float32)
    for i in range(N_CORES):
        r = res.results[i]["hout"]            # [2, 128, HC, T, b_core] fp16
        # h[d, t, b, hc*128 + p] = r[d, p, hc, t, b]; bwd already in original time
        h = r.astype(np.float32).transpose(0, 3, 4, 2, 1).reshape(2, T, B_CORE, H)
        out[i * B_CORE:(i + 1) * B_CORE, :, 0:H] = h[0].transpose(1, 0, 2)
        out[i * B_CORE:(i + 1) * B_CORE, :, H:2 * H] = h[1].transpose(1, 0, 2)
    return out, res


def kernel(**inputs):
    return run(inputs, trace=False)[0]


# revision 16
# speedup vs baseline: 1.0683x; 1.0201x over previous
"""Trainium2 Bass kernel for a Tacotron-style encoder:
   embedding -> 3x (conv1d k=5 SAME + BN + ReLU) -> bidirectional LSTM (zoneout, eval).

Contract: kernel(**inputs) takes FULL unsharded inputs (as numpy arrays) and
returns the FULL [B, T, 2H] float32 output. Internally shards batch across 8
NeuronCores (data-parallel), runs a Bass/Tile kernel per core, and gathers.

Self-contained: hardcodes all shapes; does not read sibling files.

v3: fp16 front-end in (time, batch)-blocked layout (contiguous evictions),
single per-layer conv weight DMAs, recurrence with 32 segments/direction,
warm=16, software-pipelined emission across the two direction groups.
"""

import numpy as np
import ml_dtypes
BF = ml_dtypes.bfloat16

import concourse.bacc as bacc
import concourse.bass as bass
import concourse.tile as tile
from concourse import mybir
from concourse.bass_utils import run_bass_kernel_spmd

# Model dims (hardcoded from the problem spec)
B, T, V, E, H, F, K = 32, 512, 256, 512, 256, 512, 5
ZONEOUT = 0.1
BN_EPS = 1e-3
N_CORES = 8
B_CORE = B // N_CORES  # 4

F32 = mybir.dt.float32
F16 = mybir.dt.float16
BF16 = mybir.dt.bfloat16

# Gate chunk permutation: Keras order (i, f, g, o) -> device order (i, f, o, g)
_GATE_PERM = np.r_[0:2 * H, 3 * H:4 * H, 2 * H:3 * H]

# Recurrence config
SEG = 32       # segments per direction
WARM = 18      # warmup steps per segment


def build_program(Tn=T, b=B_CORE, seg=SEG, warm=WARM):
    """Build the per-core Bass program. Returns the Bacc object."""
    nc = bacc.Bacc(trn_type="TRN2", debug=False, num_devices=N_CORES)

    n_core = b * Tn
    EC = E // 128   # 4 embedding-dim chunks
    FC = F // 128   # 4 feature chunks
    VC = V // 128   # 2 vocab chunks
    GC = 4 * H // 128  # 8 gate chunks
    HC = H // 128   # 2 hidden chunks
    sl = Tn // seg        # segment length
    nslot = sl + warm     # recurrence slots per direction-group
    TP = Tn + warm        # padded xw time axis
    TB = Tn // 128        # 128-step time blocks for the front-end

    # ---- DRAM I/O (per core) ----
    tok_d = nc.dram_tensor("tokens", [n_core], F32, kind="ExternalInput")
    viota_d = nc.dram_tensor("viota", [128, VC], F32, kind="ExternalInput")
    ident_d = nc.dram_tensor("ident", [128, 128], F16, kind="ExternalInput")
    embw_d = nc.dram_tensor("embw", [128, VC, EC, 128], F16, kind="ExternalInput")
    convw_d = nc.dram_tensor("convw", [3, 128, FC, FC, K, 128], F16, kind="ExternalInput")
    cbias_d = nc.dram_tensor("cbias", [128, 3 * FC], F32, kind="ExternalInput")
    wx_d = nc.dram_tensor("wx", [128, 2, FC, GC, 128], F16, kind="ExternalInput")
    wh_d = nc.dram_tensor("wh", [128, 2, HC, GC, 128], F16, kind="ExternalInput")
    lbias_d = nc.dram_tensor("lbias", [128, 2 * GC], F32, kind="ExternalInput")
    hout_d = nc.dram_tensor("hout", [2, 128, HC, Tn, b], F16, kind="ExternalOutput")

    sig = mybir.ActivationFunctionType.Sigmoid
    tanh = mybir.ActivationFunctionType.Tanh
    mult = mybir.AluOpType.mult
    add = mybir.AluOpType.add

    with tile.TileContext(nc) as tc:
        with tc.tile_pool(name="const", bufs=1) as const, \
             tc.tile_pool(name="lstmw", bufs=1) as lstmw, \
             tc.tile_pool(name="xwp", bufs=1) as xwp, \
             tc.tile_pool(name="hbuf", bufs=1) as hbuf:

            cb = const.tile([128, 3 * FC], F32)
            nc.sync.dma_start(out=cb[:], in_=cbias_d.ap())
            lb = const.tile([128, 2 * GC], F32)
            nc.sync.dma_start(out=lb[:], in_=lbias_d.ap())
            ident = const.tile([128, 128], F16)
            nc.sync.dma_start(out=ident[:], in_=ident_d.ap())
            wh_sb = lstmw.tile([128, 2, HC, GC, 128], F16)
            nc.sync.dma_start(out=wh_sb[:], in_=wh_d.ap())
            viota = const.tile([128, VC], F32)
            nc.sync.dma_start(out=viota[:], in_=viota_d.ap())

            psb_cm = tc.tile_pool(name="psb", bufs=8, space="PSUM")
            psb = psb_cm.__enter__()

            # x layout: [128, FC, Tn+4 (time, SAME pad 2+2), b]
            with tc.tile_pool(name="xp", bufs=2) as xp:
                def fresh_x():
                    xt = xp.tile([128, FC, Tn + 4, b], F16, tag="x")
                    nc.vector.memset(xt[:, :, 0:2, :], 0.0)
                    nc.vector.memset(xt[:, :, Tn + 2:Tn + 4, :], 0.0)
                    return xt

                # ---- embedding via one-hot matmul (tokens fed t-major) ----
                with tc.tile_pool(name="embp", bufs=1) as embp:
                    embw = embp.tile([128, VC, EC, 128], F16)
                    nc.sync.dma_start(out=embw[:], in_=embw_d.ap())
                    tokb = embp.tile([128, n_core], F32)
                    tok_ap = tok_d.ap()
                    nc.sync.dma_start(
                        out=tokb[:],
                        in_=bass.AP(tensor=tok_ap.tensor, offset=0,
                                    ap=[[0, 128]] + list(tok_ap.ap)),
                    )
                    oh = embp.tile([128, VC, n_core], F16)
                    for vc in range(VC):
                        nc.vector.tensor_scalar(
                            out=oh[:, vc, :], in0=tokb[:], scalar1=viota[:, vc:vc + 1],
                            scalar2=None, op0=mybir.AluOpType.is_equal,
                        )
                    x0 = fresh_x()
                    for mc in range(EC):
                        ps4 = [psb.tile([128, 128, b], F32, tag="ps",
                                        name=f"pse{tb}") for tb in range(TB)]
                        for vc in range(VC):
                            for tb in range(TB):
                                nc.tensor.matmul(
                                    out=ps4[tb][:],
                                    lhsT=embw[:, vc, mc, :],
                                    rhs=oh[:, vc, tb * 128 * b:(tb + 1) * 128 * b],
                                    start=(vc == 0), stop=(vc == VC - 1),
                                    skip_group_check=True,
                                )
                        for tb in range(TB):
                            nc.scalar.activation(
                                out=x0[:, mc, 2 + tb * 128:2 + (tb + 1) * 128, :],
                                in_=ps4[tb][:], func=mybir.ActivationFunctionType.Copy,
                            )

                # ---- 3 conv layers (BN folded; ReLU+bias fused on eviction) ----
                xcur = x0
                with tc.tile_pool(name="cwp", bufs=2) as cwp:
                    for l in range(3):
                        wl = cwp.tile([128, FC, FC, K, 128], F16, tag="wl")
                        nc.sync.dma_start(out=wl[:], in_=convw_d.ap()[l])
                        xn = fresh_x()
                        for mc in range(FC):
                            ps4 = [psb.tile([128, 128, b], F32, tag="ps",
                                            name=f"psc{tb}") for tb in range(TB)]
                            nmm = FC * K
                            i = 0
                            for kc in range(FC):
                                for k in range(K):
                                    for tb in range(TB):
                                        nc.tensor.matmul(
                                            out=ps4[tb][:],
                                            lhsT=wl[:, mc, kc, k, :],
                                            rhs=xcur[:, kc, tb * 128 + k:tb * 128 + k + 128, :],
                                            start=(i == 0), stop=(i == nmm - 1),
                                            skip_group_check=True,
                                        )
                                    i += 1
                            for tb in range(TB):
                                nc.scalar.activation(
                                    out=xn[:, mc, 2 + tb * 128:2 + (tb + 1) * 128, :],
                                    in_=ps4[tb][:],
                                    func=mybir.ActivationFunctionType.Relu,
                                    bias=cb[:, l * FC + mc:l * FC + mc + 1],
                                )
                        xcur = xn

                # ---- LSTM input projections xw = x @ Wx + b, padded time ----
                # d=0 (fwd): time t at index warm + t; pad [0:warm) = 0
                # d=1 (bwd): time t at index t; pad [Tn:TP) = 0
                with tc.tile_pool(name="wxp", bufs=1) as wxp:
                    wx_sb = wxp.tile([128, 2, FC, GC, 128], F16)
                    nc.sync.dma_start(out=wx_sb[:], in_=wx_d.ap())
                    xw = []
                    for d in range(2):
                        xwd = xwp.tile([128, GC, TP, b], F16, tag=f"xw{d}",
                                       name=f"xw{d}")
                        if d == 0:
                            nc.vector.memset(xwd[:, :, 0:warm, :], 0.0)
                        else:
                            nc.vector.memset(xwd[:, :, Tn:TP, :], 0.0)
                        base = warm if d == 0 else 0
                        for mc in range(GC):
                            ps4 = [psb.tile([128, 128, b], F32, tag="ps",
                                            name=f"psw{tb}") for tb in range(TB)]
                            for kc in range(FC):
                                for tb in range(TB):
                                    nc.tensor.matmul(
                                        out=ps4[tb][:],
                                        lhsT=wx_sb[:, d, kc, mc, :],
                                        rhs=xcur[:, kc, 2 + tb * 128:2 + (tb + 1) * 128, :],
                                        start=(kc == 0), stop=(kc == FC - 1),
                                        skip_group_check=True,
                                    )
                            for tb in range(TB):
                                nc.scalar.activation(
                                    out=xwd[:, mc, base + tb * 128:base + (tb + 1) * 128, :],
                                    in_=ps4[tb][:],
                                    func=mybir.ActivationFunctionType.Identity,
                                    bias=lb[:, d * GC + mc:d * GC + mc + 1],
                                )
                        xw.append(xwd)

            psb_cm.__exit__(None, None, None)

            # ---- recurrence: 2 direction-groups x `seg` chains ----
            # group d, col-slice s, lane bb at slot j:
            #   d=0 processes t = sl*s + (j - warm); d=1: t = (sl-1) + warm - j + sl*s
            # Chains reading the zero pad keep exactly-zero state, so the
            # first in-range step starts from the true initial condition.
            h_sb = hbuf.tile([128, 2, HC, Tn, b], F16, name="h_sb")

            with tc.tile_pool(name="state", bufs=3) as stp, \
                 tc.tile_pool(name="ew", bufs=2) as ew, \
                 tc.tile_pool(name="psg", bufs=2, space="PSUM") as psg:

                st = []
                for d in range(2):
                    c0 = stp.tile([128, HC, seg, b], F32, tag=f"C{d}", name=f"C0_{d}")
                    nc.vector.memset(c0[:], 0.0)
                    h0 = stp.tile([128, HC, seg, b], F16, tag=f"H{d}", name=f"H0_{d}")
                    nc.vector.memset(h0[:], 0.0)
                    st.append({"C": c0, "H": h0})

                def mm_block(d, j):
                    s = st[d]
                    ps = psg.tile([128, GC, seg, b], F32, tag=f"ps{d}")
                    off = j if d == 0 else (sl - 1) + warm - j
                    xv = xw[d][:, :, off:off + (seg - 1) * sl + 1:sl, :]
                    for mc in range(GC):
                        nc.tensor.matmul(
                            out=ps[:, mc, :, :], lhsT=ident[:], rhs=xv[:, mc, :, :],
                            start=True, stop=False, skip_group_check=True)
                        for kc in range(HC):
                            nc.tensor.matmul(
                                out=ps[:, mc, :, :], lhsT=wh_sb[:, d, kc, mc, :],
                                rhs=s["H"][:, kc, :, :],
                                start=False, stop=(kc == HC - 1),
                                skip_group_check=True)
                    s["ps"] = ps

                def acts(d):
                    s = st[d]
                    S = ew.tile([128, 6, seg, b], F16, tag=f"S{d}")
                    nc.scalar.activation(out=S[:], in_=s["ps"][:, 0:6, :, :], func=sig)
                    Tg = ew.tile([128, HC, seg, b], F16, tag=f"Tg{d}")
                    nc.scalar.activation(out=Tg[:], in_=s["ps"][:, 6:8, :, :], func=tanh)
                    s["S"], s["Tg"] = S, Tg

                def vchain(d):
                    s = st[d]
                    m2 = ew.tile([128, HC, seg, b], F32, tag=f"m2{d}")
                    nc.vector.tensor_tensor(out=m2[:], in0=s["S"][:, 2:4, :, :],
                                            in1=s["C"][:], op=mult)
                    m1 = ew.tile([128, HC, seg, b], F16, tag=f"m1{d}")
                    nc.vector.tensor_tensor(out=m1[:], in0=s["S"][:, 0:2, :, :],
                                            in1=s["Tg"][:], op=mult)
                    cn = ew.tile([128, HC, seg, b], F32, tag=f"cn{d}")
                    nc.vector.scalar_tensor_tensor(
                        out=cn[:], in0=m2[:], scalar=1.0 - ZONEOUT, in1=m1[:],
                        op0=mult, op1=add)
                    s["m2"], s["cn"] = m2, cn

                def tc_act(d):
                    s = st[d]
                    TC = ew.tile([128, HC, seg, b], F16, tag=f"TC{d}")
                    nc.scalar.activation(out=TC[:], in_=s["cn"][:], func=tanh)
                    s["TC"] = TC

                def cn_update(d):
                    s = st[d]
                    Cn = stp.tile([128, HC, seg, b], F32, tag=f"C{d}", name=f"Cn{d}")
                    nc.vector.scalar_tensor_tensor(
                        out=Cn[:], in0=s["C"][:], scalar=ZONEOUT, in1=s["cn"][:],
                        op0=mult, op1=add)
                    s["C"] = Cn

                def h_update(d, j):
                    s = st[d]
                    if j >= warm:
                        po = (j - warm) if d == 0 else (sl - 1) - (j - warm)
                        hview = h_sb[:, d, :, po:po + (seg - 1) * sl + 1:sl, :]
                    else:
                        hw = ew.tile([128, HC, seg, b], F16, tag=f"hw{d}")
                        hview = hw[:]
                    nc.vector.tensor_tensor(out=hview, in0=s["S"][:, 4:6, :, :],
                                            in1=s["TC"][:], op=mult)
                    Hn = stp.tile([128, HC, seg, b], F16, tag=f"H{d}", name=f"Hn{d}")
                    nc.vector.scalar_tensor_tensor(
                        out=Hn[:], in0=s["H"][:], scalar=ZONEOUT, in1=hview,
                        op0=mult, op1=add)
                    s["H"] = Hn

                # software-pipelined emission across the two groups
                for j in range(nslot):
                    mm_block(0, j)
                    acts(0)
                    mm_block(1, j)
                    vchain(0)
                    acts(1)
                    tc_act(0)
                    vchain(1)
                    h_update(0, j)
                    tc_act(1)
                    cn_update(0)
                    h_update(1, j)
                    cn_update(1)

            for d in range(2):
                nc.sync.dma_start(out=hout_d.ap()[d], in_=h_sb[:, d, :, :, :])

    nc.compile()
    return nc


def prep_weights(emb, conv_w, conv_b, bn_gamma, bn_beta, bn_mean, bn_var,
                 lstm_wx, lstm_wh, lstm_b):
    """Host-side weight folding + layout. Returns dict of device arrays."""
    EC, FC, VC = E // 128, F // 128, V // 128
    GC, HC = 4 * H // 128, H // 128

    inv = bn_gamma / np.sqrt(bn_var + BN_EPS)              # [3, F]
    dev = {}
    dev["embw"] = np.ascontiguousarray(
        emb.reshape(VC, 128, EC, 128).transpose(1, 0, 2, 3)).astype(np.float16)

    cw = np.empty((3, 128, FC, FC, K, 128), np.float16)
    cbias = np.empty((128, 3 * FC), np.float32)
    for l in range(3):
        wf = conv_w[l] * inv[l][None, None, :]             # [K, F, F]
        # [K, FC_in, 128_in, FC_out, 128_out] -> [128_in, FC_out, FC_in, K, 128_out]
        cw[l] = wf.reshape(K, FC, 128, FC, 128).transpose(2, 3, 1, 0, 4)
        bf = (conv_b[l] - bn_mean[l]) * inv[l] + bn_beta[l]  # [F]
        cbias[:, l * FC:(l + 1) * FC] = bf.reshape(FC, 128).T
    dev["convw"] = cw
    dev["cbias"] = cbias

    wx = np.empty((128, 2, FC, GC, 128), np.float16)
    wh = np.empty((128, 2, HC, GC, 128), np.float16)
    lbias = np.empty((128, 2 * GC), np.float32)
    for d in range(2):
        wxp = lstm_wx[d][:, _GATE_PERM]                    # [F, 4H]
        wx[:, d] = wxp.reshape(FC, 128, GC, 128).transpose(1, 0, 2, 3)
        whp = (1.0 - ZONEOUT) * lstm_wh[d][:, _GATE_PERM]  # [H, 4H]
        wh[:, d] = whp.reshape(HC, 128, GC, 128).transpose(1, 0, 2, 3).astype(np.float16)
        lbias[:, d * GC:(d + 1) * GC] = lstm_b[d][_GATE_PERM].reshape(GC, 128).T
    dev["wx"] = wx
    dev["wh"] = wh
    dev["lbias"] = lbias
    dev["viota"] = np.arange(V, dtype=np.float32).reshape(VC, 128).T.copy()
    dev["ident"] = np.eye(128, dtype=np.float16)
    return dev


_CACHED_NC = None


def _get_nc():
    global _CACHED_NC
    if _CACHED_NC is None:
        _CACHED_NC = build_program()
    return _CACHED_NC


def run(inputs, trace=False, **spmd_kwargs):
    """Run on 8 cores. Returns (output [B, T, 2H] f32, BassKernelResults)."""
    nc = _get_nc()
    dev = prep_weights(
        inputs["emb"], inputs["conv_w"], inputs["conv_b"], inputs["bn_gamma"],
        inputs["bn_beta"], inputs["bn_mean"], inputs["bn_var"],
        inputs["lstm_wx"], inputs["lstm_wh"], inputs["lstm_b"])
    tokens = np.asarray(inputs["tokens"], np.int32)

    in_maps = []
    for i in range(N_CORES):
        m = dict(dev)
        # t-major per core: col index = t * b + lane
        m["tokens"] = np.ascontiguousarray(
            tokens[i * B_CORE:(i + 1) * B_CORE].T.reshape(-1).astype(np.float32))
        in_maps.append(m)

    res = run_bass_kernel_spmd(nc, in_maps, core_ids=list(range(N_CORES)),
                               trace=trace, **spmd_kwargs)

    out = np.empty((B, T, 2 * H), np.float32)
    for i in range(N_CORES):
        r = res.results[i]["hout"]            # [2, 128, HC, T, b_core] fp16
        # h[d, t, b, hc*128 + p] = r[d, p, hc, t, b]; bwd already in original time
        h = r.astype(np.float32).transpose(0, 3, 4, 2, 1).reshape(2, T, B_CORE, H)
        out[i * B_CORE:(i + 1) * B_CORE, :, 0:H] = h[0].transpose(1, 0, 2)
        out[i * B_CORE:(i + 1) * B_CORE, :, H:2 * H] = h[1].transpose(1, 0, 2)
    return out, res


def kernel(**inputs):
    return run(inputs, trace=False)[0]
